# revision 1
# baseline (speedup 1.0000x reference)
"""Binarized linear kernel for Trainium2 (8 NeuronCores, SPMD).

Computes out = x @ sign(weight).T with
  x:      [8192, 4096] f32
  weight: [4096, 4096] f32
  out:    [8192, 4096] f32

Strategy (data-parallel over M, per the sharding hint's second option):
  - Host: cast x/weight to bf16 (sign() is exact under the cast; x loses
    <2^-9 relative, well inside the matmul tolerance), pre-transpose both
    so the contraction dim K lands on SBUF partitions without any
    on-device transposes (fp32/bf16 transposed loads are slow paths).
  - Each core c gets xT shard [K, 1024] (columns c*1024:(c+1)*1024 of
    xT) resident in SBUF, and streams the full wT [K, 4096] once,
    binarizing tiles on the Scalar engine (Sign activation) on the fly.
  - PE: for each (n_tile of 512, m_tile of 128): accumulate 32 matmuls
    (K=4096 in chunks of 128) into one PSUM bank, evict via DVE copy,
    DMA the [128, 512] f32 block to the output shard.
  - Gather: concatenate the 8 per-core [1024, 4096] outputs over M.
"""

import os
import sys

import numpy as np

# Toolchain locations (normally already on sys.path via PYTHONPATH; be
# robust when invoked from a fresh directory/environment).
for _p in (
    "/root/.axon_site",
    "/root/.axon_site/_ro/trn_rl_repo",
    "/root/.axon_site/_ro/pypackages",
    "/opt/trn_rl_repo",
):
    if os.path.isdir(_p) and _p not in sys.path:
        sys.path.append(_p)

import ml_dtypes  # noqa: E402

BF16 = ml_dtypes.bfloat16

M, K, N = 8192, 4096, 4096
N_CORES = 8
P = 128
N_TILE = 512


def build_nc(mc: int = M // N_CORES, k: int = K, n: int = N):
    """Build the per-core Bass program. Same program runs SPMD on all
    cores; only the input data differs."""
    from concourse import bacc, mybir, tile

    ko_cnt = k // P
    mj_cnt = mc // P
    nt_cnt = n // N_TILE

    nc = bacc.Bacc("TRN2", target_bir_lowering=False)

    xT = nc.dram_tensor("xT", [k, mc], mybir.dt.bfloat16, kind="ExternalInput")
    wT = nc.dram_tensor("wT", [k, n], mybir.dt.bfloat16, kind="ExternalInput")
    out = nc.dram_tensor("out", [mc, n], mybir.dt.float32, kind="ExternalOutput")

    xT_ap = xT[:].rearrange("(ko p) m -> p ko m", p=P)
    wT_ap = wT[:].rearrange("(ko p) n -> p ko n", p=P)
    out_ap = out[:].rearrange("(t p) n -> t p n", p=P)

    n_warm = 20 if mc >= 1024 else 0

    with tile.TileContext(nc) as tc:
        with (
            tc.tile_pool(name="xres", bufs=1) as xpool,
            tc.tile_pool(name="warmp", bufs=1) as warmpool,
            tc.tile_pool(name="w", bufs=3) as wpool,
            tc.tile_pool(name="o", bufs=4) as opool,
            tc.tile_pool(name="ps", bufs=8, space="PSUM") as pspool,
        ):
            # HAM warm-up: dummy matmuls on a zeroed tile fill the ~12us
            # of dead PE time while the prologue + first DMAs run, so the
            # real matmul stream starts at 2.4 GHz instead of 1.2.
            if n_warm:
                warm = warmpool.tile([P, N_TILE], mybir.dt.bfloat16)
                nc.gpsimd.memset(warm[:], 0)
                warm_ps = pspool.tile([P, N_TILE], mybir.dt.float32, tag="ps")
                for _ in range(n_warm):
                    nc.tensor.matmul(
                        warm_ps[:], warm[:, :P], warm[:], start=True, stop=True
                    )

            x_res = xpool.tile([P, ko_cnt, mc], mybir.dt.bfloat16)

            def load_w(nt, interleave_x=False):
                w_tile = wpool.tile([P, ko_cnt, N_TILE], mybir.dt.bfloat16)
                nsl = slice(nt * N_TILE, (nt + 1) * N_TILE)
                for ko in range(ko_cnt):
                    if interleave_x:
                        if ko < 2:
                            # Halve the first chunks: lower arrival latency
                            # for the very first matmuls during the ramp.
                            h = mc // 2
                            nc.sync.dma_start(x_res[:, ko, :h], xT_ap[:, ko, :h])
                            nc.sync.dma_start(x_res[:, ko, h:], xT_ap[:, ko, h:])
                        else:
                            nc.sync.dma_start(x_res[:, ko, :], xT_ap[:, ko, :])
                    if interleave_x and ko < 2:
                        h = N_TILE // 2
                        n0 = nt * N_TILE
                        nc.sync.dma_start(
                            w_tile[:, ko, :h], wT_ap[:, ko, n0 : n0 + h]
                        )
                        nc.sync.dma_start(
                            w_tile[:, ko, h:], wT_ap[:, ko, n0 + h : n0 + N_TILE]
                        )
                    else:
                        nc.sync.dma_start(w_tile[:, ko, :], wT_ap[:, ko, nsl])
                    # Binarize in place: bf16 {-1, 0, +1}; exact values.
                    nc.scalar.sign(w_tile[:, ko, :], w_tile[:, ko, :])
                return w_tile

            # First n-tile's weight stream is interleaved with the x
            # residency load so the PE can start as early as possible.
            w0 = load_w(0, interleave_x=True)

            for nt in range(nt_cnt):
                w_tile = w0 if nt == 0 else load_w(nt)
                nsl = slice(nt * N_TILE, (nt + 1) * N_TILE)
                if nt == 0:
                    # k-outer during the ramp: one (x, w) chunk-pair per
                    # k-step feeds 8 matmuls (one per psum bank), so the
                    # PE keeps up with the DMA arrival order.
                    pss = [
                        pspool.tile(
                            [P, N_TILE], mybir.dt.float32, name=f"ps0_{mj}", tag="ps"
                        )
                        for mj in range(mj_cnt)
                    ]
                    for ko in range(ko_cnt):
                        for mj in range(mj_cnt):
                            nc.tensor.matmul(
                                pss[mj][:],
                                x_res[:, ko, mj * P : (mj + 1) * P],
                                w_tile[:, ko, :],
                                start=(ko == 0),
                                stop=(ko == ko_cnt - 1),
                            )
                    for mj in range(mj_cnt):
                        o_t = opool.tile([P, N_TILE], mybir.dt.float32)
                        nc.vector.tensor_copy(out=o_t[:], in_=pss[mj][:])
                        nc.sync.dma_start(out_ap[mj, :, nsl], o_t[:])
                    continue
                for mj in range(mj_cnt):
                    ps = pspool.tile([P, N_TILE], mybir.dt.float32, tag="ps")
                    o_t = opool.tile([P, N_TILE], mybir.dt.float32)
                    if nt == nt_cnt - 1 and mj == mj_cnt - 1:
                        # Kernel-tail drain: run the final tile as two
                        # sequential N=256 accumulation groups, so the
                        # first half's copy + store complete under the
                        # second half's matmuls and only 128KB remains
                        # after the last matmul.
                        h = N_TILE // 2
                        n0 = nt * N_TILE
                        for half in range(2):
                            hs = slice(half * h, (half + 1) * h)
                            for ko in range(ko_cnt):
                                nc.tensor.matmul(
                                    ps[:, hs],
                                    x_res[:, ko, mj * P : (mj + 1) * P],
                                    w_tile[:, ko, hs],
                                    start=(ko == 0),
                                    stop=(ko == ko_cnt - 1),
                                )
                            nc.vector.tensor_copy(out=o_t[:, hs], in_=ps[:, hs])
                            nc.sync.dma_start(
                                out_ap[mj, :, n0 + half * h : n0 + (half + 1) * h],
                                o_t[:, hs],
                            )
                    else:
                        for ko in range(ko_cnt):
                            nc.tensor.matmul(
                                ps[:],
                                x_res[:, ko, mj * P : (mj + 1) * P],
                                w_tile[:, ko, :],
                                start=(ko == 0),
                                stop=(ko == ko_cnt - 1),
                            )
                        nc.vector.tensor_copy(out=o_t[:], in_=ps[:])
                        nc.sync.dma_start(out_ap[mj, :, nsl], o_t[:])

    return nc


_CACHE: dict = {}


def _get_finalized_nc():
    nc = _CACHE.get("nc")
    if nc is None:
        nc = build_nc()
        nc.finalize()
        _CACHE["nc"] = nc
    return nc


def _host_prep(x: np.ndarray, weight: np.ndarray):
    """bf16 cast + K-major transposes. Returns (xT_global [8*K, mc], wT)."""
    mc = M // N_CORES
    # bf16 transposes through uint16 views (vectorized; ml_dtypes object
    # paths can be slow for strided copies).
    x_u16 = np.ascontiguousarray(
        x.astype(BF16).view(np.uint16).reshape(N_CORES, mc, K).transpose(0, 2, 1)
    )
    xt_global = x_u16.reshape(N_CORES * K, mc).view(BF16)
    wt = np.ascontiguousarray(weight.astype(BF16).view(np.uint16).T).view(BF16)
    return xt_global, wt


def make_in_maps(x: np.ndarray, weight: np.ndarray):
    xt_global, wt = _host_prep(x, weight)
    return [
        {"xT": xt_global[c * K : (c + 1) * K], "wT": wt} for c in range(N_CORES)
    ]


def kernel(x: np.ndarray, weight: np.ndarray) -> np.ndarray:
    x = np.asarray(x)
    weight = np.asarray(weight)
    assert x.shape == (M, K) and weight.shape == (N, K)

    nc = _get_finalized_nc()
    from concourse.bass_utils import run_bass_kernel_spmd

    in_maps = make_in_maps(x, weight)
    try:
        res = run_bass_kernel_spmd(nc, in_maps, core_ids=list(range(N_CORES)))
    except Exception:
        # Transient device hiccups (e.g. NRT_EXEC_UNIT_UNRECOVERABLE) have
        # been observed once across many runs; one retry clears them.
        res = run_bass_kernel_spmd(nc, in_maps, core_ids=list(range(N_CORES)))
    out = np.concatenate([res.results[c]["out"] for c in range(N_CORES)], axis=0)
    return np.ascontiguousarray(out.astype(np.float32, copy=False))



# revision 4
# speedup vs baseline: 1.2531x; 1.2531x over previous
"""Binarized linear kernel for Trainium2 (8 NeuronCores, SPMD).

Computes out = x @ sign(weight).T with
  x:      [8192, 4096] f32
  weight: [4096, 4096] f32
  out:    [8192, 4096] f32

Strategy (data-parallel over M; mixed-precision contraction):
  - sign(weight) is exactly representable in fp8-e4m3 and bf16, so the
    weight binarization happens on the host for free and the device just
    streams pre-signed weights.
  - The PE's fp8 DoubleRow mode contracts 256 rows per 512-cycle matmul
    (2x the bf16 rate). Quantizing all of x to e4m3 costs 2.66e-2 rel
    error (over the 2e-2 budget), so only NFP8=14 of the 32 k-chunks of
    128 use the fp8 path (x in e4m3) and the remaining 18 stay bf16:
    total rel err = 2.66e-2 * sqrt(14/32) ~= 1.75e-2, while PE time per
    output tile drops from 32 to 7 (DoubleRow pairs) + 18 = 25 matmuls.
  - Each core keeps its x shard resident in SBUF (fp8 + bf16 copies,
    K-major so the contraction dim is on partitions) and streams the
    shared weights once per n-column-pair of 1024: n-tiles are processed
    in PAIRS so each stationary x-chunk load (LDWEIGHTS) feeds two
    matmuls back-to-back, halving weight-load pressure on the PE.
  - Ramp: 20 dummy warm-up matmuls un-throttle the HAM clock gate while
    the first DMAs land; the first n-pair runs k-outer over mj-groups of
    4 so matmuls chase the DMA arrival order; the very last psum bank is
    drained as two half-width accumulation groups to shrink the tail.
"""

import os
import sys

import numpy as np

# Toolchain locations (normally already on sys.path via PYTHONPATH; be
# robust when invoked from a fresh directory/environment).
for _p in (
    "/root/.axon_site",
    "/root/.axon_site/_ro/trn_rl_repo",
    "/root/.axon_site/_ro/pypackages",
    "/opt/trn_rl_repo",
):
    if os.path.isdir(_p) and _p not in sys.path:
        sys.path.append(_p)

import ml_dtypes  # noqa: E402

BF16 = ml_dtypes.bfloat16
FP8 = ml_dtypes.float8_e4m3  # IEEE-style e4m3: matches TRN FP8_EXP4 for |v|<=240

M, K, N = 8192, 4096, 4096
N_CORES = 8
P = 128
N_TILE = 512

NFP8 = 14  # k-chunks (of 128) contracted in fp8 DoubleRow; must be even
NBF = K // P - NFP8  # k-chunks contracted in bf16
KF8 = NFP8 * P  # leading k rows in fp8


def build_nc(mc: int = M // N_CORES, n: int = N):
    """Per-core Bass program (SPMD: same program, different data)."""
    from concourse import bacc, mybir, tile

    DR = mybir.MatmulPerfMode.DoubleRow
    npair = NFP8 // 2
    mj_cnt = mc // P
    np_cnt = n // (2 * N_TILE)  # n-tile PAIRS of 1024 columns

    nc = bacc.Bacc("TRN2", target_bir_lowering=False)

    x8T = nc.dram_tensor("x8T", [KF8, mc], mybir.dt.float8e4, kind="ExternalInput")
    xbT = nc.dram_tensor("xbT", [K - KF8, mc], mybir.dt.bfloat16, kind="ExternalInput")
    w8T = nc.dram_tensor("w8T", [KF8, n], mybir.dt.float8e4, kind="ExternalInput")
    wbT = nc.dram_tensor("wbT", [K - KF8, n], mybir.dt.bfloat16, kind="ExternalInput")
    out = nc.dram_tensor("out", [mc, n], mybir.dt.float32, kind="ExternalOutput")

    x8_ap = x8T[:].rearrange("(ko p) m -> p ko m", p=P)
    xb_ap = xbT[:].rearrange("(ko p) m -> p ko m", p=P)
    w8_ap = w8T[:].rearrange("(ko p) n -> p ko n", p=P)
    wb_ap = wbT[:].rearrange("(ko p) n -> p ko n", p=P)
    out_ap = out[:].rearrange("(t p) n -> t p n", p=P)

    with tile.TileContext(nc) as tc:
        with (
            tc.tile_pool(name="xres", bufs=1) as xpool,
            tc.tile_pool(name="warmp", bufs=1) as warmpool,
            tc.tile_pool(name="w", bufs=2) as wpool,
            tc.tile_pool(name="o", bufs=6) as opool,
            tc.tile_pool(name="ps", bufs=8, space="PSUM") as pspool,
        ):
            # HAM warm-up: dummy matmuls on a zeroed tile fill the dead
            # PE time while the prologue + first DMAs run, so the real
            # matmul stream starts at 2.4 GHz instead of 1.2.
            warm = warmpool.tile([P, N_TILE], mybir.dt.bfloat16)
            nc.gpsimd.memset(warm[:], 0)
            warm_ps = pspool.tile([P, N_TILE], mybir.dt.float32, tag="ps")
            for _ in range(20):
                nc.tensor.matmul(warm_ps[:], warm[:, :P], warm[:], start=True, stop=True)

            x8_res = xpool.tile([P, NFP8, mc], mybir.dt.float8e4)
            xb_res = xpool.tile([P, NBF, mc], mybir.dt.bfloat16)

            def load_w(g, interleave_x=False):
                """Load the weight pair-block for n-cols [g*1024, (g+1)*1024).
                When interleave_x, also issue the x residency loads in
                consumption order (first pair only)."""
                w8_t = wpool.tile([P, NFP8, 2 * N_TILE], mybir.dt.float8e4)
                wb_t = wpool.tile([P, NBF, 2 * N_TILE], mybir.dt.bfloat16)
                n0 = g * 2 * N_TILE
                nsl = slice(n0, n0 + 2 * N_TILE)
                for ko in range(NFP8):
                    if interleave_x:
                        if ko < 2:
                            h = mc // 2
                            nc.sync.dma_start(x8_res[:, ko, :h], x8_ap[:, ko, :h])
                            nc.sync.dma_start(x8_res[:, ko, h:], x8_ap[:, ko, h:])
                        else:
                            nc.sync.dma_start(x8_res[:, ko, :], x8_ap[:, ko, :])
                    if interleave_x and ko < 2:
                        h = N_TILE
                        nc.sync.dma_start(w8_t[:, ko, :h], w8_ap[:, ko, n0 : n0 + h])
                        nc.sync.dma_start(
                            w8_t[:, ko, h:], w8_ap[:, ko, n0 + h : n0 + 2 * h]
                        )
                    else:
                        nc.sync.dma_start(w8_t[:, ko, :], w8_ap[:, ko, nsl])
                for ko in range(NBF):
                    if interleave_x:
                        nc.sync.dma_start(xb_res[:, ko, :], xb_ap[:, ko, :])
                    nc.sync.dma_start(wb_t[:, ko, :], wb_ap[:, ko, nsl])
                return w8_t, wb_t

            def mm_steps(ps, w8_t, wb_t, mj, half, nsl_w):
                """Issue the 25 accumulation matmuls for one psum tile.
                nsl_w: slice of the w tiles' 1024 n-columns. half selects
                psum columns [0:len] (always full ps width here)."""
                msl = slice(mj * P, (mj + 1) * P)
                for j in range(npair):
                    nc.tensor.matmul(
                        ps,
                        x8_res[:, 2 * j : 2 * j + 2, msl],
                        w8_t[:, 2 * j : 2 * j + 2, nsl_w],
                        start=(j == 0),
                        stop=False,
                        perf_mode=DR,
                    )
                for ko in range(NBF):
                    nc.tensor.matmul(
                        ps,
                        xb_res[:, ko, msl],
                        wb_t[:, ko, nsl_w],
                        start=False,
                        stop=(ko == NBF - 1),
                    )

            def evict(ps_t, mj, n0, width=2 * N_TILE, o_t=None, osl=None):
                if o_t is None:
                    o_t = opool.tile([P, N_TILE], mybir.dt.float32, name="o_t")
                    osl = slice(0, N_TILE)
                nc.vector.tensor_copy(out=o_t[:, osl], in_=ps_t)
                nc.sync.dma_start(out_ap[mj, :, n0 : n0 + N_TILE], o_t[:, osl])

            # ---- n-pair 0: k-outer over mj-groups of 4 (DMA-chasing ramp)
            w8_0, wb_0 = load_w(0, interleave_x=True)
            for grp in range(2):
                pss = [
                    pspool.tile(
                        [P, N_TILE], mybir.dt.float32, name=f"ps0_{grp}_{i}", tag="ps"
                    )
                    for i in range(8)
                ]
                for j in range(npair):
                    for i in range(4):
                        mj = grp * 4 + i
                        msl = slice(mj * P, (mj + 1) * P)
                        for nt in range(2):
                            nc.tensor.matmul(
                                pss[2 * i + nt][:],
                                x8_res[:, 2 * j : 2 * j + 2, msl],
                                w8_0[:, 2 * j : 2 * j + 2, nt * N_TILE : (nt + 1) * N_TILE],
                                start=(j == 0),
                                stop=False,
                                perf_mode=DR,
                            )
                for ko in range(NBF):
                    for i in range(4):
                        mj = grp * 4 + i
                        msl = slice(mj * P, (mj + 1) * P)
                        for nt in range(2):
                            nc.tensor.matmul(
                                pss[2 * i + nt][:],
                                xb_res[:, ko, msl],
                                wb_t0_slice := wb_0[:, ko, nt * N_TILE : (nt + 1) * N_TILE],
                                start=False,
                                stop=(ko == NBF - 1),
                            )
                for i in range(4):
                    mj = grp * 4 + i
                    for nt in range(2):
                        evict(pss[2 * i + nt][:], mj, nt * N_TILE)

            # ---- n-pairs 1..3: mj-outer, stationary reused across the pair
            for g in range(1, np_cnt):
                w8_t, wb_t = load_w(g)
                n0 = g * 2 * N_TILE
                for mj in range(mj_cnt):
                    last = g == np_cnt - 1 and mj == mj_cnt - 1
                    ps_a = pspool.tile(
                        [P, N_TILE], mybir.dt.float32, name=f"psa_{g}_{mj}", tag="ps"
                    )
                    if not last:
                        ps_b = pspool.tile(
                            [P, N_TILE], mybir.dt.float32, name=f"psb_{g}_{mj}", tag="ps"
                        )
                        msl = slice(mj * P, (mj + 1) * P)
                        for j in range(npair):
                            for nt, ps in ((0, ps_a), (1, ps_b)):
                                nc.tensor.matmul(
                                    ps[:],
                                    x8_res[:, 2 * j : 2 * j + 2, msl],
                                    w8_t[:, 2 * j : 2 * j + 2, nt * N_TILE : (nt + 1) * N_TILE],
                                    start=(j == 0),
                                    stop=False,
                                    perf_mode=DR,
                                )
                        for ko in range(NBF):
                            for nt, ps in ((0, ps_a), (1, ps_b)):
                                nc.tensor.matmul(
                                    ps[:],
                                    xb_res[:, ko, msl],
                                    wb_t[:, ko, nt * N_TILE : (nt + 1) * N_TILE],
                                    start=False,
                                    stop=(ko == NBF - 1),
                                )
                        evict(ps_a[:], mj, n0)
                        evict(ps_b[:], mj, n0 + N_TILE)
                    else:
                        # Kernel-tail drain: nt0 normally, then nt1 as two
                        # sequential half-width groups so the first half's
                        # copy + store complete under the second half's
                        # matmuls.
                        mm_steps(ps_a[:], w8_t, wb_t, mj, 0, slice(0, N_TILE))
                        evict(ps_a[:], mj, n0)
                        ps_b = pspool.tile(
                            [P, N_TILE], mybir.dt.float32, name=f"psbl_{g}_{mj}", tag="ps"
                        )
                        o_t = opool.tile([P, N_TILE], mybir.dt.float32)
                        h = N_TILE // 2
                        msl = slice(mj * P, (mj + 1) * P)
                        for half in range(2):
                            hsl = slice(N_TILE + half * h, N_TILE + (half + 1) * h)
                            psl = slice(half * h, (half + 1) * h)
                            for j in range(npair):
                                nc.tensor.matmul(
                                    ps_b[:, psl],
                                    x8_res[:, 2 * j : 2 * j + 2, msl],
                                    w8_t[:, 2 * j : 2 * j + 2, hsl],
                                    start=(j == 0),
                                    stop=False,
                                    perf_mode=DR,
                                )
                            for ko in range(NBF):
                                nc.tensor.matmul(
                                    ps_b[:, psl],
                                    xb_res[:, ko, msl],
                                    wb_t[:, ko, hsl],
                                    start=False,
                                    stop=(ko == NBF - 1),
                                )
                            nc.vector.tensor_copy(out=o_t[:, psl], in_=ps_b[:, psl])
                            nc.sync.dma_start(
                                out_ap[mj, :, n0 + N_TILE + half * h : n0 + N_TILE + (half + 1) * h],
                                o_t[:, psl],
                            )

    return nc


_CACHE: dict = {}


def _get_finalized_nc():
    nc = _CACHE.get("nc")
    if nc is None:
        nc = build_nc()
        nc.finalize()
        _CACHE["nc"] = nc
    return nc


def _host_prep(x: np.ndarray, weight: np.ndarray):
    """Pre-sign weights, split-cast x, K-major transposes.

    Returns (x8_global [8*KF8, mc] fp8, xb_global [8*(K-KF8), mc] bf16,
             w8T [KF8, N] fp8, wbT [K-KF8, N] bf16)."""
    mc = M // N_CORES
    wb = np.sign(weight).astype(np.float32, copy=False)
    wT8 = np.ascontiguousarray(wb[:, :KF8].astype(FP8).view(np.uint8).T).view(FP8)
    wTb = (
        np.ascontiguousarray(wb[:, KF8:].astype(BF16).view(np.uint16).T).view(BF16)
    )
    # x: per-core K-major shards (transpose via integer views: ml_dtypes
    # object paths are slow for strided copies).
    x8 = np.ascontiguousarray(
        x[:, :KF8].astype(FP8).view(np.uint8).reshape(N_CORES, mc, KF8).transpose(0, 2, 1)
    )
    x8_global = x8.reshape(N_CORES * KF8, mc).view(FP8)
    xb = np.ascontiguousarray(
        x[:, KF8:].astype(BF16).view(np.uint16).reshape(N_CORES, mc, K - KF8).transpose(0, 2, 1)
    )
    xb_global = xb.reshape(N_CORES * (K - KF8), mc).view(BF16)
    return x8_global, xb_global, wT8, wTb


def make_in_maps(x: np.ndarray, weight: np.ndarray):
    x8_global, xb_global, wT8, wTb = _host_prep(x, weight)
    kb = K - KF8
    return [
        {
            "x8T": x8_global[c * KF8 : (c + 1) * KF8],
            "xbT": xb_global[c * kb : (c + 1) * kb],
            "w8T": wT8,
            "wbT": wTb,
        }
        for c in range(N_CORES)
    ]


def kernel(x: np.ndarray, weight: np.ndarray) -> np.ndarray:
    x = np.asarray(x)
    weight = np.asarray(weight)
    assert x.shape == (M, K) and weight.shape == (N, K)

    nc = _get_finalized_nc()
    from concourse.bass_utils import run_bass_kernel_spmd

    in_maps = make_in_maps(x, weight)
    try:
        res = run_bass_kernel_spmd(nc, in_maps, core_ids=list(range(N_CORES)))
    except Exception:
        # Transient device hiccups (e.g. NRT_EXEC_UNIT_UNRECOVERABLE) have
        # been observed once across many runs; one retry clears them.
        res = run_bass_kernel_spmd(nc, in_maps, core_ids=list(range(N_CORES)))
    out = np.concatenate([res.results[c]["out"] for c in range(N_CORES)], axis=0)
    return np.ascontiguousarray(out.astype(np.float32, copy=False))
